# revision 11
# baseline (speedup 1.0000x reference)
"""HAN (2-layer heterogeneous GAT) on 8 Trainium2 NeuronCores (Bass/Tile).

Single fused launch. Each core holds 1/8 of the nodes (dst-slice sharding
per edge type). Per core: project own node slice -> [h|score] row tables;
AllGather the src tables across cores (stacked-slice row layout); per-edge
dma_gather of src rows and dst rows (one-hot window mask + dst score);
alpha = leaky_relu(es+ed), w = exp(alpha); PE matmul one-hot^T @ [w*h | w]
accumulated per 128-dst window (window-major edge order), normalized +
transposed + semantic-attention tanh partials fused at flush. pw partials
AllReduce'd on device; beta + layer-2 + final combine all on device.
Host does only integer/byte marshaling before the single launch.
"""
import numpy as np
import ml_dtypes

import jax

jax.config.update("jax_compilation_cache_dir", "/tmp/jax_bass_cache")
jax.config.update("jax_persistent_cache_min_entry_size_bytes", -1)
jax.config.update("jax_persistent_cache_min_compile_time_secs", 0)

import concourse.bacc as bacc
import concourse.tile as tile
import concourse.mybir as mybir
from concourse import bass_utils

BF = ml_dtypes.bfloat16
N_A, N_P, E, NC = 50000, 100000, 800000, 8
SL_A, SL_P = N_A // NC, N_P // NC                # 6250, 12500
W_A, W_P = (SL_A + 127) // 128, (SL_P + 127) // 128  # 49, 98
PAD_A, PAD_P = W_A * 128, W_P * 128              # 6272, 12544
CHK = 32768
EPS = 1e-6

f32, bf16, i16 = mybir.dt.float32, mybir.dt.bfloat16, mybir.dt.int16
ADD, MULT, MAX = mybir.AluOpType.add, mybir.AluOpType.mult, mybir.AluOpType.max
AF = mybir.ActivationFunctionType


# ---------------------------------------------------------------- host prep --
def pack16c(idx):
    """Compact [16, n/16] i16 index layout (device replicates to 128 rows)."""
    return np.ascontiguousarray(idx.reshape(-1, 16).T.astype(np.int16))


def prep_type(src, dst, SL_d, PAD_d, SL_s, PAD_s, n_win):
    """Window-major uniform-schedule edge prep for one edge type.

    Edges sorted by (dst window, src chunk); per-key tile counts padded to
    the max across cores so one SPMD program serves all 8 cores.
    s16: chunk-relative stacked src row.  d16: local dst row (sentinel PAD_d).
    """
    n_chk = (NC * PAD_s + CHK - 1) // CHK
    K = n_win * n_chk
    per = []
    for c in range(NC):
        m = (dst >= c * SL_d) & (dst < (c + 1) * SL_d)
        es = src[m].astype(np.int64)
        er = (es // SL_s) * PAD_s + (es % SL_s)      # stacked row
        ed = (dst[m] - c * SL_d).astype(np.int64)
        key = (ed >> 7) * n_chk + er // CHK
        o = np.argsort(key, kind="stable")
        per.append((er[o], ed[o], key[o]))
    cnts = np.stack([np.bincount(p[2], minlength=K) for p in per])
    T = (cnts.max(0) + 127) // 128
    # guarantee every window emits at least one tile (all-sentinel is fine)
    for w in range(n_win):
        if T[w * n_chk:(w + 1) * n_chk].sum() == 0:
            T[w * n_chk] = 1
    offs = np.zeros(K + 1, np.int64)
    offs[1:] = np.cumsum(T) * 128
    n_tiles = int(T.sum())
    npad = n_tiles * 128
    sched = []
    for w in range(n_win):
        segs = [(ch, int(T[w * n_chk + ch]))
                for ch in range(n_chk) if T[w * n_chk + ch] > 0]
        sched.append(segs)
    s16, d16 = [], []
    for er, ed, key in per:
        sa = np.zeros(npad, np.int64)
        da = np.full(npad, PAD_d, np.int64)
        st, cn = np.unique(key, return_index=True)
        cnt = np.diff(np.append(cn, len(key)))
        for k, s0, c_ in zip(st, cn, cnt):
            off = offs[k]
            sa[off:off + c_] = er[s0:s0 + c_] - (k % n_chk) * CHK
            da[off:off + c_] = ed[s0:s0 + c_]
        s16.append(pack16c(sa))
        d16.append(pack16c(da))
    return dict(n_tiles=n_tiles, sched=sched, s16=s16, d16=d16, n_chk=n_chk)


def ablk(a, F):
    H = a.shape[0]
    o = np.zeros((F, H), np.float32)
    for h in range(H):
        o[h * 16:(h + 1) * 16, h] = a[h]
    return o


def padTb(x, SL, PAD, c):
    """Transposed bf16 slice [F, PAD] of rows [c*SL:(c+1)*SL]."""
    o = np.zeros((x.shape[1], PAD), BF)
    o[:, :SL] = x[c * SL:(c + 1) * SL].T
    return np.ascontiguousarray(o)


# ------------------------------------------------------------ device pieces --
def build_wa(nc, pool, psum, cp, WT_d, W_d, brow_d, bcol_d, A_ds,
             kin, fout, hw, tag):
    """rhs = [W (kin,fout) | W@A_i (kin,hw)...] f32 + brep (128,nrhs)."""
    nA = len(A_ds)
    nrhs = fout + hw * nA
    WT = pool.tile([128, kin], f32, tag="bwt")
    nc.sync.dma_start(WT[0:fout, :], WT_d[:])
    WTb = pool.tile([128, kin], bf16, tag="bwtb")
    nc.vector.tensor_copy(WTb[0:fout, :], WT[0:fout, :])
    rhs = cp.tile([128, nrhs], f32, tag="rhs" + tag)
    Wn = pool.tile([128, fout], f32, tag="bwn")
    nc.sync.dma_start(Wn[0:kin, :], W_d[:])
    nc.vector.tensor_copy(rhs[:, 0:fout], Wn[:])
    bx = cp.tile([1, nrhs], f32, tag="bx" + tag)
    bn = pool.tile([1, fout], f32, tag="bbn")
    nc.sync.dma_start(bn[:], brow_d[:])
    nc.vector.tensor_copy(bx[:, 0:fout], bn[:])
    bc = pool.tile([128, 1], f32, tag="bbc")
    nc.sync.dma_start(bc[0:fout, :], bcol_d[:])
    for i, A_d in enumerate(A_ds):
        Ab = pool.tile([128, hw], f32, tag="bab")
        nc.sync.dma_start(Ab[0:fout, :], A_d[:])
        Abb = pool.tile([128, hw], bf16, tag="babb")
        nc.vector.tensor_copy(Abb[0:fout, :], Ab[0:fout, :])
        ps = psum.tile([128, hw], f32, tag="ps")
        nc.tensor.matmul(ps[0:kin, :], WTb[0:fout, 0:kin], Abb[0:fout, :],
                         start=True, stop=True)
        nc.vector.tensor_copy(rhs[:, fout + hw * i:fout + hw * (i + 1)],
                              ps[0:kin, :])
        psb = psum.tile([1, hw], f32, tag="ps")
        nc.tensor.matmul(psb[:], bc[0:fout, 0:1], Ab[0:fout, :], start=True,
                         stop=True)
        nc.vector.tensor_copy(bx[:, fout + hw * i:fout + hw * (i + 1)],
                              psb[:])
    rhsb = cp.tile([128, nrhs], bf16, tag="rhsb" + tag)
    nc.vector.tensor_copy(rhsb[:], rhs[:])
    ones = cp.tile([1, 128], f32, tag="ones" + tag)
    nc.gpsimd.memset(ones[:], 1.0)
    bps = psum.tile([128, nrhs], f32, tag="ps")
    nc.tensor.matmul(bps[:], ones[:], bx[:], start=True, stop=True)
    brep = cp.tile([128, nrhs], f32, tag="brep" + tag)
    nc.vector.tensor_copy(brep[:], bps[:])
    return rhs, rhsb, brep, ones


def emit_proj1(nc, pool, psum, spool, xT_d, rhs_list, brep, nrhs, n_tiles,
               F, S, src_tbl, dst_tbl, dS):
    """L1 projection of this core's slice: stage [h|scores] rows into
    src_tbl and dst scores into dst_tbl's f32 cols [64:64+dS]."""
    st = [None]
    dt_ = [None]
    s3 = src_tbl[:].rearrange("(c r) e -> r c e", r=128)
    d3f = dst_tbl[:].bitcast(f32).rearrange("(c r) e -> r c e", r=128)
    for c0 in range(0, n_tiles, 8):
        ntc = min(8, n_tiles - c0)
        xbb = pool.tile([128, ntc * 128], bf16, tag="pxb")
        nc.sync.dma_start(xbb[:], xT_d[:, c0 * 128:(c0 + ntc) * 128])
        for t in range(ntc):
            gt = c0 + t
            tl = gt % 16
            if tl == 0:
                st[0] = spool.tile([128, 16, 256], bf16, tag="stage",
                                   name="stage")
                dt_[0] = spool.tile([128, 16, dS], f32, tag="dstage",
                                    name="dstage")
            ps = psum.tile([128, nrhs], f32, tag="ps")
            for i, rh in enumerate(rhs_list):
                nc.tensor.matmul(ps[:], xbb[:, t * 128:(t + 1) * 128],
                                 rh[:], start=(i == 0),
                                 stop=(i == len(rhs_list) - 1))
            nc.vector.tensor_tensor(st[0][:, tl, 0:F], ps[:, 0:F],
                                    brep[:, 0:F], op=ADD)
            nc.vector.tensor_tensor(
                st[0][:].bitcast(f32)[:, tl, 64:64 + S],
                ps[:, F:F + S], brep[:, F:F + S], op=ADD)
            nc.vector.tensor_tensor(dt_[0][:, tl, :], ps[:, F + S:nrhs],
                                    brep[:, F + S:nrhs], op=ADD)
            if tl == 15 or gt == n_tiles - 1:
                cc = gt - tl
                nc.sync.dma_start(s3[:, cc:cc + tl + 1, :],
                                  st[0][:, 0:tl + 1, :])
                nc.sync.dma_start(d3f[:, cc:cc + tl + 1, 64:64 + dS],
                                  dt_[0][:, 0:tl + 1, :])


def emit_proj2(nc, pool, psum, spool, oT_ds, rhs_list, brep, nrhs, n_tiles,
               F, S, src_tbl, dst_tbl, dS):
    """L2 projection from transposed o tables; [h2|es] rows (256B) into
    src_tbl, dst scores into dst_tbl f32 cols [64:64+dS] (if dS)."""
    st = [None]
    dt_ = [None]
    s3 = src_tbl[:].rearrange("(c r) e -> r c e", r=128)
    d3f = (dst_tbl[:].bitcast(f32).rearrange("(c r) e -> r c e", r=128)
           if dS else None)
    for c0 in range(0, n_tiles, 8):
        ntc = min(8, n_tiles - c0)
        xbbs = []
        for oT_d in oT_ds:
            xbb = pool.tile([128, ntc * 128], bf16, tag="pxb")
            nc.sync.dma_start(xbb[:], oT_d[:, c0 * 128:(c0 + ntc) * 128])
            xbbs.append(xbb)
        for t in range(ntc):
            gt = c0 + t
            tl = gt % 16
            if tl == 0:
                st[0] = spool.tile([128, 16, 128], bf16, tag="stage2",
                                   name="stage2")
                if dS:
                    dt_[0] = spool.tile([128, 16, dS], f32, tag="dstage2",
                                        name="dstage2")
            ps = psum.tile([128, nrhs], f32, tag="ps")
            for i, xbb in enumerate(xbbs):
                nc.tensor.matmul(ps[:], xbb[:, t * 128:(t + 1) * 128],
                                 rhs_list[i][:], start=(i == 0),
                                 stop=(i == len(xbbs) - 1))
            nc.vector.tensor_tensor(st[0][:, tl, 0:F], ps[:, 0:F],
                                    brep[:, 0:F], op=ADD)
            nc.vector.tensor_tensor(
                st[0][:].bitcast(f32)[:, tl, 32:32 + S],
                ps[:, F:F + S], brep[:, F:F + S], op=ADD)
            if dS:
                nc.vector.tensor_tensor(dt_[0][:, tl, :], ps[:, F + S:nrhs],
                                        brep[:, F + S:nrhs], op=ADD)
            if tl == 15 or gt == n_tiles - 1:
                cc = gt - tl
                nc.sync.dma_start(s3[:, cc:cc + tl + 1, :],
                                  st[0][:, 0:tl + 1, :])
                if dS:
                    nc.sync.dma_start(d3f[:, cc:cc + tl + 1, 64:64 + dS],
                                      dt_[0][:, 0:tl + 1, :])


def emit_edge(nc, pool, psume, psum2, src_tbl, dst_tbl, s16d, d16d, meta,
              F, H, sset, dset, src_rows, oT_tbl=None, o_rows=None,
              tanh_ctx=None, ident=None, rep_gather=1, rep_mm=1):
    """Window-major edge phase: gather, alpha, exp, one-hot matmul psum,
    fused normalize (+relu) -> transpose -> table write (+tanh partial)."""
    NR = F + H
    selem = 256 if F == 128 else 128
    so = (64 if F == 128 else 32) + sset * H
    off = 0
    for w, segs in enumerate(meta["sched"]):
        ntw = sum(t for _, t in segs)
        si = pool.tile([128, ntw * 8], i16, tag="si")
        di = pool.tile([128, ntw * 8], i16, tag="di")
        nc.sync.dma_start(si[:], s16d[:, off * 8:(off + ntw) * 8])
        nc.sync.dma_start(di[:], d16d[:, off * 8:(off + ntw) * 8])
        G = pool.tile([128, ntw, selem], bf16, tag="G")
        D = pool.tile([128, ntw, 256], bf16, tag="D")
        for _r in range(rep_gather):
            t0 = 0
            for qi, (ch, tn) in enumerate(segs):
                b = ch * CHK
                nc.gpsimd.dma_gather(
                    out_ap=G[:, t0:t0 + tn, :],
                    in_ap=src_tbl[b:min(b + CHK, src_rows), :],
                    idxs_ap=si[:, t0 * 8:(t0 + tn) * 8],
                    num_idxs=tn * 128, num_idxs_reg=tn * 128,
                    elem_size=selem, single_packet=False)
                t0 += tn
            nc.gpsimd.dma_gather(
                out_ap=D[:, 0:ntw, :], in_ap=dst_tbl[:], idxs_ap=di[:],
                num_idxs=ntw * 128, num_idxs_reg=ntw * 128, elem_size=256,
                single_packet=False)
        Gf, Df = G[:].bitcast(f32), D[:].bitcast(f32)
        al = pool.tile([128, ntw, H], f32, tag="al")
        nc.vector.tensor_tensor(al[:], Gf[:, 0:ntw, so:so + H],
                                Df[:, 0:ntw, 64 + dset * H:64 + (dset + 1) * H],
                                op=ADD)
        lr = pool.tile([128, ntw, H], f32, tag="lr")
        nc.vector.tensor_scalar(out=lr[:], in0=al[:], scalar1=0.2,
                                scalar2=None, op0=MULT)
        nc.vector.tensor_tensor(lr[:], lr[:], al[:], op=MAX)
        wex = pool.tile([128, ntw, H], f32, tag="wex")
        nc.scalar.activation(wex[:], lr[:], AF.Exp)
        VW = pool.tile([128, ntw, NR], bf16, tag="VW")
        nc.vector.tensor_tensor(
            VW[:, :, 0:F].rearrange("p t (h d) -> p t h d", h=H),
            G[:, 0:ntw, 0:F].rearrange("p t (h d) -> p t h d", h=H),
            wex[:, :, :, None].broadcast_to([128, ntw, H, 16]), op=MULT)
        nc.vector.tensor_copy(VW[:, :, F:NR], wex[:])
        ep = psume.tile([128, NR], f32, tag="eps", name="eps")
        for _r in range(rep_mm):
            for t in range(ntw):
                nc.tensor.matmul(ep[:], D[:, t, 0:128], VW[:, t, :],
                                 start=(t == 0 and _r == 0),
                                 stop=(t == ntw - 1 and _r == rep_mm - 1))
        # flush: normalize + relu
        rc = pool.tile([128, H], f32, tag="rc")
        nc.vector.tensor_scalar(out=rc[:], in0=ep[:, F:NR], scalar1=EPS,
                                scalar2=None, op0=ADD)
        nc.vector.reciprocal(rc[:], rc[:])
        ot = pool.tile([128, F], f32, tag="ot")
        nc.vector.tensor_tensor(
            ot[:].rearrange("p (h d) -> p h d", h=H),
            ep[:, 0:F].rearrange("p (h d) -> p h d", h=H),
            rc[:, :, None].broadcast_to([128, H, 16]), op=MULT)
        nc.vector.tensor_scalar(out=ot[:], in0=ot[:], scalar1=0.0,
                                scalar2=None, op0=MAX)
        if o_rows is not None:
            nc.sync.dma_start(o_rows[w * 128:(w + 1) * 128, :], ot[:])
        if oT_tbl is not None or tanh_ctx is not None:
            tp = psum2.tile([128, 128], f32, tag="tps", name="tps")
            nc.tensor.transpose(tp[0:F, :], ot[:], ident[:])
            oTb = pool.tile([128, 128], bf16, tag="oTb")
            nc.vector.tensor_copy(oTb[0:F, :], tp[0:F, :])
            if oT_tbl is not None:
                nc.sync.dma_start(oT_tbl[:, w * 128:(w + 1) * 128],
                                  oTb[0:F, :])
            if tanh_ctx is not None:
                Wkb, bk, qb, qacc = tanh_ctx
                ps2 = psum2.tile([128, 128], f32, tag="tps", name="tps2")
                nc.tensor.matmul(ps2[0:F, :], Wkb[0:F, 0:F], oTb[0:F, :],
                                 start=True, stop=True)
                th = pool.tile([128, 128], bf16, tag="th")
                nc.scalar.activation(th[0:F, :], ps2[0:F, :], AF.Tanh,
                                     bias=bk[0:F, :])
                ps3 = psum2.tile([1, 128], f32, tag="tps3", name="tps3")
                nc.tensor.matmul(ps3[:], qb[0:F, 0:1], th[0:F, :],
                                 start=True, stop=True)
                nc.vector.tensor_tensor(qacc[:], qacc[:], ps3[:], op=ADD)
        off += ntw


def make_tanh_ctx(nc, cp, pool, psum, Wk_d, bk_d, q_d, F, n_pad_rows, tag):
    """Load Wk/bk/q, return (ctx, corr) where corr = -n_pad * q.tanh(bk)."""
    Wkf = pool.tile([128, F], f32, tag="wkf")
    nc.sync.dma_start(Wkf[0:F, :], Wk_d[:])
    Wkb = cp.tile([128, F], bf16, tag="wkb" + tag)
    nc.vector.tensor_copy(Wkb[0:F, :], Wkf[0:F, :])
    bk = cp.tile([128, 1], f32, tag="bk" + tag)
    nc.sync.dma_start(bk[0:F, :], bk_d[:])
    qf = cp.tile([128, 1], f32, tag="qf" + tag)
    nc.sync.dma_start(qf[0:F, :], q_d[:])
    qb = cp.tile([128, 1], bf16, tag="qb" + tag)
    nc.vector.tensor_copy(qb[0:F, :], qf[0:F, :])
    tb = pool.tile([128, 1], f32, tag="tbk")
    nc.scalar.activation(tb[0:F, :], bk[0:F, :], AF.Tanh)
    corr_ps = psum.tile([1, 1], f32, tag="ps")
    nc.tensor.matmul(corr_ps[:], qf[0:F, 0:1], tb[0:F, :], start=True,
                     stop=True)
    corr = cp.tile([1, 1], f32, tag="corr" + tag)
    nc.vector.tensor_scalar(out=corr[:], in0=corr_ps[:],
                            scalar1=-float(n_pad_rows), scalar2=None,
                            op0=MULT)
    return Wkb, bk, qb, corr


def emit_beta(nc, pool, psum, pw, n_nodes, ones):
    """pw: [1,2] summed partial scores -> beta columns [128,1] x2."""
    s = pool.tile([1, 2], f32, tag="pt2")
    nc.vector.tensor_scalar(out=s[:], in0=pw[:], scalar1=1.0 / n_nodes,
                            scalar2=None, op0=MULT)
    e = pool.tile([1, 2], f32, tag="pt3")
    nc.scalar.activation(e[:], s[:], AF.Exp)
    dn = pool.tile([1, 1], f32, tag="pt4")
    nc.vector.tensor_reduce(dn[:], e[:], axis=mybir.AxisListType.X, op=ADD)
    rcp = pool.tile([1, 1], f32, tag="pt5")
    nc.vector.reciprocal(rcp[:], dn[:])
    beta = pool.tile([1, 2], f32, tag="pt6")
    nc.vector.tensor_tensor(beta[:], e[:], rcp[:].broadcast_to([1, 2]),
                            op=MULT)
    cols = []
    for m in range(2):
        ps = psum.tile([128, 1], f32, tag="ps")
        nc.tensor.matmul(ps[:], ones[:], beta[0:1, m:m + 1], start=True,
                         stop=True)
        col = pool.tile([128, 1], f32, tag=f"bcol{m}")
        nc.vector.tensor_copy(col[:], ps[:])
        cols.append(col)
    return cols


# ----------------------------------------------------------------- kernel ---
def build_fused(meta):
    nc = bacc.Bacc(None, target_bir_lowering=False, debug=False,
                   num_devices=NC)
    dt = nc.dram_tensor
    I, O, N = "ExternalInput", "ExternalOutput", "Internal"
    xTa = dt("xTa", [128, PAD_A], bf16, kind=I)
    xTp = dt("xTp", [128, PAD_P], bf16, kind=I)
    W1a = dt("W1a", [128, 128], f32, kind=I)
    W1aT = dt("W1aT", [128, 128], f32, kind=I)
    W1p = dt("W1p", [128, 128], f32, kind=I)
    W1pT = dt("W1pT", [128, 128], f32, kind=I)
    b1ar = dt("b1ar", [1, 128], f32, kind=I)
    b1ac = dt("b1ac", [128, 1], f32, kind=I)
    b1pr = dt("b1pr", [1, 128], f32, kind=I)
    b1pc = dt("b1pc", [128, 1], f32, kind=I)
    A = {k: dt("A" + k, [128, 8], f32, kind=I)
         for k in ("sap", "dap", "spa", "dpa", "saa", "daa")}
    eye_d = dt("eye", [128, 128], bf16, kind=I)
    Wk1 = dt("Wk1", [128, 128], f32, kind=I)
    bk1 = dt("bk1", [128, 1], f32, kind=I)
    q1 = dt("q1", [128, 1], f32, kind=I)
    W2a = dt("W2a", [128, 64], f32, kind=I)
    W2aT = dt("W2aT", [64, 128], f32, kind=I)
    W2p = dt("W2p", [128, 64], f32, kind=I)
    W2pT = dt("W2pT", [64, 128], f32, kind=I)
    b2ar = dt("b2ar", [1, 64], f32, kind=I)
    b2ac = dt("b2ac", [64, 1], f32, kind=I)
    b2pr = dt("b2pr", [1, 64], f32, kind=I)
    b2pc = dt("b2pc", [64, 1], f32, kind=I)
    A2 = {k: dt("A2" + k, [64, 4], f32, kind=I)
          for k in ("spa", "dpa", "saa", "daa")}
    Wk2 = dt("Wk2", [64, 64], f32, kind=I)
    bk2 = dt("bk2", [64, 1], f32, kind=I)
    q2 = dt("q2", [64, 1], f32, kind=I)
    sc, dc = {}, {}
    for ty in ("ap", "pa", "aa"):
        X = meta[ty]["n_tiles"] * 8
        sc[ty] = dt("s16c" + ty, [16, X], i16, kind=I)
        dc[ty] = dt("d16c" + ty, [16, X], i16, kind=I)
    out = dt("out", [PAD_A, 64], bf16, kind=O)

    # internal DRAM
    s16, d16 = {}, {}
    for ty in ("ap", "pa", "aa"):
        X = meta[ty]["n_tiles"] * 8
        s16[ty] = dt("s16" + ty, [128, X], i16, kind=N)
        d16[ty] = dt("d16" + ty, [128, X], i16, kind=N)
    au_loc = dt("au_loc", [PAD_A, 256], bf16, kind=N)
    pa_loc = dt("pa_loc", [PAD_P, 256], bf16, kind=N)
    au_t = dt("au_t", [NC * PAD_A, 256], bf16, kind=N, addr_space="Shared")
    pa_t = dt("pa_t", [NC * PAD_P, 256], bf16, kind=N, addr_space="Shared")
    aud_t = dt("aud_t", [PAD_A + 128, 256], bf16, kind=N)
    apd_t = dt("apd_t", [PAD_P + 128, 256], bf16, kind=N)
    oTap = dt("oTap", [128, PAD_P], bf16, kind=N)
    oTpa = dt("oTpa", [128, PAD_A], bf16, kind=N)
    oTaa = dt("oTaa", [128, PAD_A], bf16, kind=N)
    pw1l = dt("pw1l", [1, 16], f32, kind=N)
    pw1g = dt("pw1g", [1, 16], f32, kind=N, addr_space="Shared")
    au2_loc = dt("au2_loc", [PAD_A, 128], bf16, kind=N)
    pa2_loc = dt("pa2_loc", [PAD_P, 128], bf16, kind=N)
    au2_t = dt("au2_t", [NC * PAD_A, 128], bf16, kind=N, addr_space="Shared")
    pa2_t = dt("pa2_t", [NC * PAD_P, 128], bf16, kind=N, addr_space="Shared")
    aud2_t = dt("aud2_t", [PAD_A + 128, 256], bf16, kind=N)
    o2pa = dt("o2pa", [PAD_A, 64], f32, kind=N)
    o2aa = dt("o2aa", [PAD_A, 64], f32, kind=N)
    pw2l = dt("pw2l", [1, 16], f32, kind=N)
    pw2g = dt("pw2g", [1, 16], f32, kind=N, addr_space="Shared")

    RG = [list(range(NC))]
    with tile.TileContext(nc) as tc:
        with (tc.tile_pool(name="c", bufs=1) as cp,
              tc.tile_pool(name="s", bufs=2) as pool,
              tc.tile_pool(name="st", bufs=2) as spool,
              tc.tile_pool(name="p", bufs=2, space="PSUM") as psum,
              tc.tile_pool(name="pe", bufs=2, space="PSUM") as psume,
              tc.tile_pool(name="p2", bufs=1, space="PSUM") as psum2):
            # replicate compact idx tables to 128 partitions in DRAM
            for ty in ("ap", "pa", "aa"):
                for k in range(8):
                    nc.sync.dma_start(s16[ty][16 * k:16 * (k + 1), :],
                                      sc[ty][:])
                    nc.sync.dma_start(d16[ty][16 * k:16 * (k + 1), :],
                                      dc[ty][:])
            # constants
            eye = cp.tile([128, 128], bf16)
            nc.sync.dma_start(eye[:], eye_d[:])
            idf = cp.tile([128, 128], f32)
            nc.vector.tensor_copy(idf[:], eye[:])
            zrow = cp.tile([1, 256], bf16)
            nc.gpsimd.memset(zrow[:], 0.0)
            # dst tables: one-hot blocks + zero sentinel row
            for tb_, wn in ((aud_t, W_A), (apd_t, W_P), (aud2_t, W_A)):
                t3 = tb_[0:wn * 128, :].rearrange("(c r) e -> r c e", r=128)
                for w in range(wn):
                    nc.sync.dma_start(t3[:, w, 0:128], eye[:])
                nc.sync.dma_start(tb_[wn * 128:wn * 128 + 1, :], zrow[:])

            # ---- layer 1 projection (own slice only)
            _, ra, bra, ones = build_wa(nc, pool, psum, cp, W1aT, W1a, b1ar,
                                        b1ac, [A["sap"], A["saa"], A["dpa"],
                                               A["daa"]], 128, 128, 8, "a")
            _, rp, brp, _ = build_wa(nc, pool, psum, cp, W1pT, W1p, b1pr,
                                     b1pc, [A["spa"], A["dap"]],
                                     128, 128, 8, "p")
            emit_proj1(nc, pool, psum, spool, xTa, [ra], bra, 160, W_A,
                       128, 16, au_loc, aud_t, 16)
            emit_proj1(nc, pool, psum, spool, xTp, [rp], brp, 144, W_P,
                       128, 8, pa_loc, apd_t, 8)

            # ---- all-gather projected src tables
            nc.gpsimd.collective_compute(
                "AllGather", mybir.AluOpType.bypass, replica_groups=RG,
                ins=[au_loc[:].opt()], outs=[au_t[:].opt()])
            nc.gpsimd.collective_compute(
                "AllGather", mybir.AluOpType.bypass, replica_groups=RG,
                ins=[pa_loc[:].opt()], outs=[pa_t[:].opt()])

            # ---- layer 1 edge phases
            ctx1 = make_tanh_ctx(nc, cp, pool, psum, Wk1, bk1, q1, 128,
                                 PAD_A - SL_A, "1")
            Wkb1, bkt1, qb1, corr1 = ctx1
            qacc_pa = cp.tile([1, 128], f32, tag="qacc_pa")
            nc.gpsimd.memset(qacc_pa[:], 0.0)
            qacc_aa = cp.tile([1, 128], f32, tag="qacc_aa")
            nc.gpsimd.memset(qacc_aa[:], 0.0)
            emit_edge(nc, pool, psume, psum2, au_t, apd_t, s16["ap"],
                      d16["ap"], meta["ap"], 128, 8, 0, 0, NC * PAD_A,
                      oT_tbl=oTap, ident=idf)
            emit_edge(nc, pool, psume, psum2, pa_t, aud_t, s16["pa"],
                      d16["pa"], meta["pa"], 128, 8, 0, 0, NC * PAD_P,
                      oT_tbl=oTpa, tanh_ctx=(Wkb1, bkt1, qb1, qacc_pa),
                      ident=idf)
            emit_edge(nc, pool, psume, psum2, au_t, aud_t, s16["aa"],
                      d16["aa"], meta["aa"], 128, 8, 1, 1, NC * PAD_A,
                      oT_tbl=oTaa, tanh_ctx=(Wkb1, bkt1, qb1, qacc_aa),
                      ident=idf)
            # pw1 partial -> all-reduce
            pwt = pool.tile([1, 16], f32, tag="pwt")
            nc.gpsimd.memset(pwt[:], 0.0)
            for m, qa in enumerate((qacc_pa, qacc_aa)):
                red = pool.tile([1, 1], f32, tag="red")
                nc.vector.tensor_reduce(red[:], qa[:],
                                        axis=mybir.AxisListType.X, op=ADD)
                nc.vector.tensor_tensor(pwt[0:1, m:m + 1], red[:], corr1[:],
                                        op=ADD)
            nc.sync.dma_start(pw1l[:], pwt[:])
            nc.gpsimd.collective_compute(
                "AllReduce", mybir.AluOpType.add, replica_groups=RG,
                ins=[pw1l[:].opt()], outs=[pw1g[:].opt()])
            pw1s = pool.tile([1, 16], f32, tag="pw1s")
            nc.sync.dma_start(pw1s[:], pw1g[:])
            bcols1 = emit_beta(nc, pool, psum, pw1s[0:1, 0:2], N_A, ones)

            # ---- layer 2 projection
            ra2, _, bra2, _ = build_wa(nc, pool, psum, cp, W2aT, W2a, b2ar,
                                       b2ac, [A2["saa"], A2["dpa"],
                                              A2["daa"]], 128, 64, 4, "a2")
            _, rp2, brp2, _ = build_wa(nc, pool, psum, cp, W2pT, W2p, b2pr,
                                       b2pc, [A2["spa"]], 128, 64, 4, "p2")
            ra2_s = []
            for m in range(2):
                rs = cp.tile([128, 76], bf16, tag=f"ra2s{m}")
                nc.scalar.activation(rs[:], ra2[:], AF.Copy,
                                     scale=bcols1[m][:])
                ra2_s.append(rs)
            emit_proj2(nc, pool, psum, spool, [oTpa, oTaa], ra2_s, bra2, 76,
                       W_A, 64, 4, au2_loc, aud2_t, 8)
            emit_proj2(nc, pool, psum, spool, [oTap], [rp2], brp2, 68,
                       W_P, 64, 4, pa2_loc, None, 0)
            nc.gpsimd.collective_compute(
                "AllGather", mybir.AluOpType.bypass, replica_groups=RG,
                ins=[au2_loc[:].opt()], outs=[au2_t[:].opt()])
            nc.gpsimd.collective_compute(
                "AllGather", mybir.AluOpType.bypass, replica_groups=RG,
                ins=[pa2_loc[:].opt()], outs=[pa2_t[:].opt()])

            # ---- layer 2 edge phases
            ctx2 = make_tanh_ctx(nc, cp, pool, psum, Wk2, bk2, q2, 64,
                                 PAD_A - SL_A, "2")
            Wkb2, bkt2, qb2, corr2 = ctx2
            qacc2_pa = cp.tile([1, 128], f32, tag="qacc2_pa")
            nc.gpsimd.memset(qacc2_pa[:], 0.0)
            qacc2_aa = cp.tile([1, 128], f32, tag="qacc2_aa")
            nc.gpsimd.memset(qacc2_aa[:], 0.0)
            emit_edge(nc, pool, psume, psum2, pa2_t, aud2_t, s16["pa"],
                      d16["pa"], meta["pa"], 64, 4, 0, 0, NC * PAD_P,
                      o_rows=o2pa, tanh_ctx=(Wkb2, bkt2, qb2, qacc2_pa),
                      ident=idf)
            emit_edge(nc, pool, psume, psum2, au2_t, aud2_t, s16["aa"],
                      d16["aa"], meta["aa"], 64, 4, 0, 1, NC * PAD_A,
                      o_rows=o2aa, tanh_ctx=(Wkb2, bkt2, qb2, qacc2_aa),
                      ident=idf)
            pwt2 = pool.tile([1, 16], f32, tag="pwt2")
            nc.gpsimd.memset(pwt2[:], 0.0)
            for m, qa in enumerate((qacc2_pa, qacc2_aa)):
                red = pool.tile([1, 1], f32, tag="red2")
                nc.vector.tensor_reduce(red[:], qa[:],
                                        axis=mybir.AxisListType.X, op=ADD)
                nc.vector.tensor_tensor(pwt2[0:1, m:m + 1], red[:],
                                        corr2[:], op=ADD)
            nc.sync.dma_start(pw2l[:], pwt2[:])
            nc.gpsimd.collective_compute(
                "AllReduce", mybir.AluOpType.add, replica_groups=RG,
                ins=[pw2l[:].opt()], outs=[pw2g[:].opt()])
            pw2s = pool.tile([1, 16], f32, tag="pw2s")
            nc.sync.dma_start(pw2s[:], pw2g[:])
            bcols2 = emit_beta(nc, pool, psum, pw2s[0:1, 0:2], N_A, ones)

            # ---- final combine
            for t in range(W_A):
                a = pool.tile([128, 64], f32, tag="ta")
                b = pool.tile([128, 64], f32, tag="tb")
                nc.sync.dma_start(a[:], o2pa[t * 128:(t + 1) * 128, :])
                nc.sync.dma_start(b[:], o2aa[t * 128:(t + 1) * 128, :])
                nc.vector.tensor_scalar(out=a[:], in0=a[:],
                                        scalar1=bcols2[0][:, 0:1],
                                        scalar2=None, op0=MULT)
                nc.vector.tensor_scalar(out=b[:], in0=b[:],
                                        scalar1=bcols2[1][:, 0:1],
                                        scalar2=None, op0=MULT)
                nc.vector.tensor_tensor(a[:], a[:], b[:], op=ADD)
                ab = pool.tile([128, 64], bf16, tag="tab")
                nc.vector.tensor_copy(ab[:], a[:])
                nc.sync.dma_start(out[t * 128:(t + 1) * 128, :], ab[:])
    nc.compile()
    return nc


# ------------------------------------------------------------------ driver --
DBG = {}
EXEC_NS = []


def _run(nc, maps):
    import time
    t0 = time.time()
    try:
        r = bass_utils.run_bass_kernel_spmd(nc, maps,
                                            core_ids=list(range(NC)),
                                            trace=False)
    except Exception:
        r = bass_utils.run_bass_kernel_spmd(nc, maps,
                                            core_ids=list(range(NC)),
                                            trace=False)
    wall = (time.time() - t0) * 1e9
    ns = getattr(r, "exec_time_ns", None)
    EXEC_NS.append(int(ns) if ns else int(wall))
    return r


def kernel(**inp):
    inp = {k: np.asarray(v) for k, v in inp.items()}
    meta = {
        "ap": prep_type(inp["ei_ap_src"], inp["ei_ap_dst"], SL_P, PAD_P,
                        SL_A, PAD_A, W_P),
        "pa": prep_type(inp["ei_pa_src"], inp["ei_pa_dst"], SL_A, PAD_A,
                        SL_P, PAD_P, W_A),
        "aa": prep_type(inp["ei_aa_src"], inp["ei_aa_dst"], SL_A, PAD_A,
                        SL_A, PAD_A, W_A),
    }
    eye = np.eye(128, dtype=BF)
    common = dict(
        W1a=inp["W1_a"], W1aT=np.ascontiguousarray(inp["W1_a"].T),
        W1p=inp["W1_p"], W1pT=np.ascontiguousarray(inp["W1_p"].T),
        b1ar=inp["b1_a"][None, :], b1ac=inp["b1_a"][:, None],
        b1pr=inp["b1_p"][None, :], b1pc=inp["b1_p"][:, None],
        Asap=ablk(inp["a1s_ap"], 128), Adap=ablk(inp["a1d_ap"], 128),
        Aspa=ablk(inp["a1s_pa"], 128), Adpa=ablk(inp["a1d_pa"], 128),
        Asaa=ablk(inp["a1s_aa"], 128), Adaa=ablk(inp["a1d_aa"], 128),
        eye=eye, Wk1=inp["Wk1"], bk1=inp["bk1"][:, None],
        q1=inp["q1"][:, None],
        W2a=inp["W2_a"], W2aT=np.ascontiguousarray(inp["W2_a"].T),
        W2p=inp["W2_p"], W2pT=np.ascontiguousarray(inp["W2_p"].T),
        b2ar=inp["b2_a"][None, :], b2ac=inp["b2_a"][:, None],
        b2pr=inp["b2_p"][None, :], b2pc=inp["b2_p"][:, None],
        A2spa=ablk(inp["a2s_pa"], 64), A2dpa=ablk(inp["a2d_pa"], 64),
        A2saa=ablk(inp["a2s_aa"], 64), A2daa=ablk(inp["a2d_aa"], 64),
        Wk2=inp["Wk2"], bk2=inp["bk2"][:, None], q2=inp["q2"][:, None])
    maps = []
    for c in range(NC):
        d = dict(common)
        d["xTa"] = padTb(inp["x_author"], SL_A, PAD_A, c)
        d["xTp"] = padTb(inp["x_paper"], SL_P, PAD_P, c)
        for ty in ("ap", "pa", "aa"):
            d["s16c" + ty] = meta[ty]["s16"][c]
            d["d16c" + ty] = meta[ty]["d16"][c]
        maps.append(d)
    nc = build_fused(meta)
    r = _run(nc, maps)
    out = np.zeros((N_A, 64), np.float32)
    for c in range(NC):
        out[c * SL_A:(c + 1) * SL_A] = r.results[c]["out"][:SL_A].astype(
            np.float32)
    DBG["res"] = r
    return out
